# revision 40
# baseline (speedup 1.0000x reference)
"""Multi-head causal attention (B=2, T=2048, C=2048, H=16, D=128) on 8 TRN2
NeuronCores.

Sharding: core = (batch, head_group): cores 0-3 -> batch 0, cores 4-7 ->
batch 1; head_group hg = core % 4 owns heads [4*hg, 4*hg+4).  Each core:
  qkv projection for its 4 heads (q/k transposed layout, v natural layout),
  multiplicative RoPE, causal attention with transposed scores, and the
  out-projection partial product over its 512 feature columns.
Host: per-batch sum of the 4 partial outputs + out bias (the "all-reduce").

The PE engine executes its queue in order, so idle gaps during the
exp-paced attention inner loop can only be filled at emission time.
Projection and out-projection work is therefore split into ~850ns units
(4 matmuls each, one PSUM bank, sequential k-accumulation) kept in a FIFO;
one unit (two for diagonal blocks, which also wait on the causal-mask
multiply) is emitted between each attention block's scores and AV matmuls,
hiding the exp latency.  The timing build software-pipelines the next
iteration's first projection chunk into the out-projection tail.  The
softmax denominator accumulates over four round-robin fp16 chains so each
DVE add depends only on the block four earlier, keeping it off the
per-block critical path.  Diagonal-strip matmuls/exps are narrowed to the
unmasked query columns.  Partial outputs are stored bf16.
"""

import math
from contextlib import ExitStack

import ml_dtypes
import numpy as np

import concourse.bass as bass
import concourse.tile as tile
from concourse import bacc, bass_isa, mybir

BF16 = mybir.dt.bfloat16
FP16 = mybir.dt.float16
FP32 = mybir.dt.float32
AF = mybir.ActivationFunctionType

B, T, C, H, D = 2, 2048, 2048, 16, 128
HPC = 4          # heads per core
P = 128
KO = C // P      # 16 k-tiles for the projection contraction
MCH = T // 512   # 4 m-chunks of 512 tokens
NJB = T // P     # 16 key blocks
NCI = T // 512   # 4 query chunks


def build_bass(iters: int = 1, timing_io: bool = False, no_pool: bool = False,
               only: str = "", force_steady: bool = False):
    del no_pool
    nc = bacc.Bacc("TRN2", target_bir_lowering=False, debug=False, num_devices=8)

    # timing_io: big tensors become Internal DRAM (contents garbage) so each
    # run ships only a tiny input/output over the axon tunnel
    def in_tensor(name, shape, dtype):
        if timing_io:
            return nc.dram_tensor(name, shape, dtype).ap()
        return nc.dram_tensor(name, shape, dtype, kind="ExternalInput").ap()

    xT_d = in_tensor("xT", [C, T], BF16)
    wT_d = in_tensor("wT", [C, 3 * HPC * D], BF16)
    bqk_d = nc.dram_tensor("bqk", [P, 2 * HPC], FP32, kind="ExternalInput").ap()
    bv_d = in_tensor("bv", [P, HPC * D], FP32)
    embq_d = in_tensor("embq", [P, T], BF16)
    embk_d = in_tensor("embk", [P, T], BF16)
    woT_d = in_tensor("woT", [HPC * D, C], BF16)
    dmask_d = in_tensor("dmask", [P, 4 * 512], FP16)
    if timing_io:
        y_d = nc.dram_tensor("y", [T, C], BF16).ap()
        yext_d = nc.dram_tensor("yext", [P, 512], BF16, kind="ExternalOutput").ap()
    else:
        y_d = nc.dram_tensor("y", [T, C], BF16, kind="ExternalOutput").ap()
        yext_d = None

    with tile.TileContext(nc) as tc, ExitStack() as ctx:
        cpool = ctx.enter_context(tc.tile_pool(name="consts", bufs=1))
        qk_pool = ctx.enter_context(tc.tile_pool(name="qk", bufs=1))
        v_pool = ctx.enter_context(tc.tile_pool(name="v", bufs=1))
        a_pool = ctx.enter_context(tc.tile_pool(name="aT", bufs=1))
        s_pool = ctx.enter_context(tc.tile_pool(name="es", bufs=8))
        r_pool = ctx.enter_context(tc.tile_pool(name="recip", bufs=8))
        rb_pool = ctx.enter_context(tc.tile_pool(name="recipb", bufs=3))
        y_pool = ctx.enter_context(tc.tile_pool(name="ys", bufs=6))
        # dedicated PSUM pools (8 banks total): filler (proj+outp) 2,
        # scores 2, AV accum 2, denominator 2
        psf_pool = ctx.enter_context(tc.tile_pool(name="psf", bufs=3, space="PSUM"))
        pss_pool = ctx.enter_context(tc.tile_pool(name="pss", bufs=3, space="PSUM"))
        pso_pool = ctx.enter_context(tc.tile_pool(name="pso", bufs=2, space="PSUM"))

        # const tiles (DMA emission deferred so xt chunk 0 leads the queue)
        w_sb = cpool.tile([P, KO, 3 * HPC * D], BF16)   # qkv weightsT
        woT_sb = cpool.tile([P, HPC, C], BF16)
        embq_sb = cpool.tile([P, T], BF16)
        embk_sb = cpool.tile([P, T], BF16)
        bqk_sb = cpool.tile([P, 2 * HPC], FP32)
        bv_sb = cpool.tile([P, HPC * D], FP32)
        # static double-buffered x tiles: chunk mc lives in xt_buf[mc % 2];
        # the DMA for chunk mc+2 is emitted at the start of attention chunk
        # mc, two chunks of lead so filler matmuls never wait on it
        xt_buf = [cpool.tile([P, KO, 512], BF16, name=f"xtbuf{i}")
                  for i in range(2)]
        dmask_sb = cpool.tile([P, 4 * 512], FP16)

        def load_consts():
            # smaller per-ko transfers so the first matmuls start early
            wTr = wT_d.rearrange("(ko p) n -> p ko n", p=P)
            for ko in range(KO):
                nc.sync.dma_start(w_sb[:, ko, :], wTr[:, ko, :])
            nc.sync.dma_start(bqk_sb[:], bqk_d)
            nc.sync.dma_start(bv_sb[:], bv_d)
            nc.sync.dma_start(embq_sb[:], embq_d)
            nc.sync.dma_start(embk_sb[:], embk_d)
            nc.sync.dma_start(woT_sb[:], woT_d.rearrange("(hh p) n -> p hh n", p=P))
            nc.sync.dma_start(dmask_sb[:], dmask_d)

        xTr = xT_d.rearrange("(ko p) m -> p ko m", p=P)

        KSUB = 4  # ko-tiles per filler unit (~850ns of PE work)

        def load_x(mc):
            nc.sync.dma_start(xt_buf[mc % 2][:], xTr[:, :, slice(mc * 512,
                                                                 (mc + 1) * 512)])

        def make_proj_units(mc, qkT, v_sb, it="", parts=(0, 1, 2)):
            """qkv projection for m-chunk mc as a list of small emit fns.
            parts selects passes: 0=q, 1=k, 2=v."""
            ms = slice(mc * 512, (mc + 1) * 512)
            xt = xt_buf[mc % 2]
            st = {}
            units = []

            for part in (p for p in (0, 1) if p in parts):
                for f in range(4):
                    fb = part * 4 + f
                    for k0 in range(0, KO, KSUB):
                        def u_qk(part=part, fb=fb, k0=k0):
                            if k0 == 0:
                                st["ps"] = psf_pool.tile(
                                    [P, 512], FP32, tag="mm",
                                    name=f"psqk{it}{mc}_{fb}")
                            ps = st["ps"]
                            for ko in range(k0, k0 + KSUB):
                                nc.tensor.matmul(
                                    ps[:],
                                    lhsT=w_sb[:, ko, fb * P:(fb + 1) * P],
                                    rhs=xt[:, ko, :],
                                    start=(ko == 0),
                                    stop=(ko == KO - 1),
                                )
                            if k0 + KSUB == KO:
                                # fused bias + multiplicative RoPE on DVE
                                emb = embq_sb if part == 0 else embk_sb
                                nc.vector.scalar_tensor_tensor(
                                    out=qkT[:, fb, ms], in0=ps[:],
                                    scalar=bqk_sb[:, fb:fb + 1],
                                    in1=emb[:, ms],
                                    op0=mybir.AluOpType.add,
                                    op1=mybir.AluOpType.mult,
                                )
                        units.append(u_qk)

            for s in range(4) if 2 in parts else ():
                for k0 in range(0, KO, KSUB):
                    def u_v(s=s, k0=k0):
                        if k0 == 0:
                            st["ps"] = psf_pool.tile(
                                [P, 512], FP32, tag="mm",
                                name=f"psv{it}{mc}_{s}")
                        ps = st["ps"]
                        for ko in range(k0, k0 + KSUB):
                            nc.tensor.matmul(
                                ps[:],
                                lhsT=xt[:, ko, s * P:(s + 1) * P],
                                rhs=w_sb[:, ko, 2 * HPC * D:3 * HPC * D],
                                start=(ko == 0),
                                stop=(ko == KO - 1),
                            )
                        if k0 + KSUB == KO:
                            nc.vector.tensor_add(
                                out=v_sb[:, mc * 4 + s, :], in0=ps[:],
                                in1=bv_sb[:],
                            )
                    units.append(u_v)
            return units

        def make_outp_units(mc, aT):
            """out-projection partial for token blocks of chunk mc."""
            units = []
            for mb in range(4 * mc, 4 * mc + 4):
                for cc in range(4):
                    def u(mb=mb, cc=cc):
                        ps = psf_pool.tile([P, 512], FP32, tag="mm",
                                           name=f"psy{mb}_{cc}")
                        for h in range(HPC):
                            nc.tensor.matmul(
                                ps[:],
                                lhsT=aT[:, h, mb * P:(mb + 1) * P],
                                rhs=woT_sb[:, h, cc * 512:(cc + 1) * 512],
                                start=(h == 0), stop=(h == HPC - 1),
                            )
                        ys = y_pool.tile([P, 512], BF16, tag="ys",
                                         name=f"ys{mb}_{cc}")
                        # alternate drain engine to split the load
                        if (mb + cc) % 2 == 0:
                            nc.vector.tensor_copy(out=ys[:], in_=ps[:])
                        else:
                            nc.scalar.copy(out=ys[:], in_=ps[:])
                        nc.sync.dma_start(
                            y_d[mb * P:(mb + 1) * P,
                                cc * 512:(cc + 1) * 512], ys[:]
                        )
                    units.append(u)
            return units

        def attn_chunk(h, ci, qkT, v_sb, aT, emit_filler):
            iss = slice(ci * 512, (ci + 1) * 512)
            njb = 4 * (ci + 1)
            ps_o = pso_pool.tile([P, 512], FP32, tag="mm", name=f"pso{h}_{ci}")
            # fp16 es + four interleaved denominator chains: block j's add
            # depends only on block j-4, so the DVE accumulation is off the
            # per-block critical path (and 16-bit ops get 2x DVE mode)
            nch = 4 if ci > 0 else 1
            acc = [r_pool.tile([P, 512], FP16, tag="acc",
                               name=f"acc{h}_{ci}_{c}") for c in range(nch)]
            es_t = [None] * njb

            def d0_of(jb):
                return max(jb * P - ci * 512, 0)

            def scores(jb):
                # scores + exp + causal mask for block jb; the matmul is
                # emitted one block ahead of the consuming AV so the exp
                # chain latency is hidden even when filler runs dry
                d0 = d0_of(jb)
                ps_s = pss_pool.tile([P, 512], FP32, tag="mm",
                                     name=f"pss{h}_{ci}_{jb}")
                nc.tensor.matmul(
                    ps_s[:, d0:],
                    lhsT=qkT[:, HPC + h, jb * P:(jb + 1) * P],
                    rhs=qkT[:, h, ci * 512 + d0:(ci + 1) * 512],
                    start=True, stop=True,
                )
                es = s_pool.tile([P, 512], FP16, tag="es", name=f"es{h}_{ci}_{jb}")
                es_t[jb] = es
                nc.scalar.activation(out=es[:, d0:], in_=ps_s[:, d0:], func=AF.Exp)
                if d0_of(jb + 1) > d0:   # diagonal block
                    dd = d0 // P
                    nc.vector.tensor_mul(
                        out=es[:, d0:], in0=es[:, d0:],
                        in1=dmask_sb[:, dd * 512 + d0:(dd + 1) * 512],
                    )

            scores(0)
            for jb in range(njb):
                d0 = d0_of(jb)
                diag = d0_of(jb + 1) > d0
                if jb + 1 < njb:
                    scores(jb + 1)
                emit_filler(2 if diag else 1, jb)
                es = es_t[jb]
                nc.tensor.matmul(
                    ps_o[:, d0:],
                    lhsT=v_sb[:, jb, h * D:(h + 1) * D],
                    rhs=es[:, d0:],
                    start=(jb == 0), stop=(jb == njb - 1),
                )
                # softmax denominator on DVE, round-robin across chains
                a = acc[jb % nch]
                if jb < nch:
                    nc.vector.tensor_copy(out=a[:], in_=es[:])
                else:
                    nc.vector.tensor_add(
                        out=a[:, d0:], in0=a[:, d0:], in1=es[:, d0:]
                    )
            if nch == 4:
                nc.vector.tensor_add(out=acc[0][:], in0=acc[0][:], in1=acc[1][:])
                nc.vector.tensor_add(out=acc[2][:], in0=acc[2][:], in1=acc[3][:])
                nc.vector.tensor_add(out=acc[0][:], in0=acc[0][:], in1=acc[2][:])
            rb = rb_pool.tile([P, 512], FP16, tag="rb", name=f"rb{h}_{ci}")
            rbr = rb_pool.tile([P, 512], FP32, tag="rbr", name=f"rbr{h}_{ci}")
            nc.gpsimd.partition_all_reduce(
                rb[:], acc[0][:], channels=P, reduce_op=bass_isa.ReduceOp.add
            )
            nc.vector.reciprocal(out=rbr[:], in_=rb[:])
            nc.vector.tensor_mul(out=aT[:, h, iss], in0=ps_o[:], in1=rbr[:])

        def body(emit_consts, steady):
            qkT = qk_pool.tile([P, 2 * HPC, T], BF16)    # q/k featT x tokens
            v_sb = v_pool.tile([P, NJB, HPC * D], BF16)  # tokens x v feats
            aT = a_pool.tile([P, HPC, T], BF16)          # attn outT per head

            filler = []   # list of (kind, fn); consumed selectively

            def add_filler(kind, units):
                filler.extend((kind, u) for u in units)

            def emit_filler(n, allowed=None):
                k = 0
                i = 0
                while k < n and i < len(filler):
                    kind, fn = filler[i]
                    if allowed is None or kind in allowed:
                        fn()
                        filler.pop(i)
                        k += 1
                    else:
                        i += 1
                return k

            def flush_filler(kinds=None):
                emit_filler(len(filler), kinds)

            if only in ("attn", "outp"):
                nc.vector.memset(qkT[:], 0.0)
                nc.vector.memset(v_sb[:], 0.0)
                if emit_consts:
                    load_consts()
            if only == "outp":
                nc.vector.memset(aT[:], 0.0)

            if only in ("", "proj") and not steady:
                load_x(0)
                load_x(1)
                if emit_consts:
                    load_consts()
                for u in make_proj_units(0, qkT, v_sb):
                    u()
            if only == "proj":
                for mc in range(1, MCH):
                    for u in make_proj_units(mc, qkT, v_sb):
                        u()
                # keep SBUF results live via a consumer
                nc.sync.dma_start(y_d[0:P, 0:512], qkT[:, 0, 0:512])
                nc.sync.dma_start(y_d[P:2 * P, 0:512], v_sb[:, 0, :])
                return
            if only == "outp":
                for mc in range(MCH):
                    for u in make_outp_units(mc, aT):
                        u()
                return

            # Filler budget: attn(mc) has 16*(mc+1) exp-latency slots.  Deferrable
            # work (out-projections, next iteration's q-projection) is saved
            # for the 64-slot attn(3); required projections (p{mc+1} before
            # attn(mc+1)) are flushed at their deadline.
            for mc in range(MCH):
                if only == "":
                    # x for chunk mc+2 (wrapping: next iteration's chunks in
                    # steady mode); its buffer's last reader finished in the
                    # previous attention chunk
                    if steady or mc + 2 < MCH:
                        load_x((mc + 2) % MCH)
                    if mc + 1 < MCH:
                        add_filler(f"p{mc + 1}",
                                   make_proj_units(mc + 1, qkT, v_sb))
                    if mc == 2 and steady:
                        # next iteration's q-projection: its only readers
                        # (attn(0) scores rhs) already ran this iteration
                        add_filler("p0q", make_proj_units(0, qkT, v_sb,
                                                          it="n", parts=(0,)))
                allowed = ({"p1"}, {"p2"}, {"p3", "o0", "p0q"},
                           {"o0", "o1", "o2", "p0q"})[mc]
                for h in range(HPC):
                    if mc == 3 and h == 3 and steady:
                        # next iteration's k/v chunk-0 projection: by h3 all
                        # other heads' chunk-0 key/value reads are emitted,
                        # and h3's own once jb>=4 (see fill below)
                        add_filler("p0kv", make_proj_units(0, qkT, v_sb,
                                                           it="n",
                                                           parts=(1, 2)))

                    def fill(n=1, jb=0, a=allowed, h=h, mc=mc):
                        if mc == 3 and h == 3 and jb >= 4:
                            a = a | {"p0kv"}
                        return emit_filler(n, a)
                    attn_chunk(h, mc, qkT, v_sb, aT, fill)
                # next chunk's projection must finish before its attention
                if mc + 1 < MCH:
                    flush_filler({f"p{mc + 1}"})
                if only == "":
                    add_filler(f"o{mc}", make_outp_units(mc, aT))
            # projection tail first: dense and independent of the last head's
            # softmax, hiding the aT latency the out-projection tail waits on
            flush_filler({"p0q", "p0kv"})
            flush_filler()
            if only == "attn":
                nc.sync.dma_start(y_d[0:P, 0:512], aT[:, 0, 0:512])

        if iters == 1 and force_steady:
            load_consts()
            load_x(0)
            load_x(1)
            body(emit_consts=False, steady=True)
        elif iters == 1:
            body(emit_consts=True, steady=False)
        else:
            load_consts()
            load_x(0)
            load_x(1)
            with tc.For_i(0, iters, 1):
                body(emit_consts=False, steady=True)
        if yext_d is not None:
            nc.sync.dma_start(yext_d[:], y_d[0:P, 0:512])

    nc.compile()
    return nc


def _rope_emb():
    freqs = np.arange(0, D, 2, dtype=np.float32) / D
    angles = np.arange(T, dtype=np.float32)[:, None] * freqs[None, :]
    return np.concatenate([np.sin(angles), np.cos(angles)], axis=-1)  # [T, D]


def make_in_maps(x, qkv_w, qkv_b, out_w):
    bf = ml_dtypes.bfloat16
    emb = _rope_emb()
    embq = np.ascontiguousarray((emb / math.sqrt(D)).T).astype(bf)  # [D, T]
    embk = np.ascontiguousarray(emb.T).astype(bf)
    # causal masks for the 4 diagonal-block offsets:
    # m[jl, d*512+il] = 1 iff il >= jl + 128*d
    jl = np.arange(P)[:, None]
    il = np.arange(512)[None, :]
    dmask = np.concatenate(
        [(il >= jl + 128 * d) for d in range(4)], axis=1
    ).astype(np.float16)
    in_maps = []
    for core in range(8):
        b, hg = divmod(core, 4)
        rq = slice(hg * 512, (hg + 1) * 512)
        rk = slice(C + hg * 512, C + (hg + 1) * 512)
        rv = slice(2 * C + hg * 512, 2 * C + (hg + 1) * 512)
        w_slice = np.concatenate([qkv_w[rq], qkv_w[rk], qkv_w[rv]], axis=0)
        wT = np.ascontiguousarray(w_slice.T).astype(bf)  # [C, 1536]
        bq = qkv_b[rq].reshape(HPC, P).T  # [128, 4] per-block columns
        bk = qkv_b[rk].reshape(HPC, P).T
        bqk = np.ascontiguousarray(np.concatenate([bq, bk], axis=1)).astype(np.float32)
        bv = np.ascontiguousarray(
            np.broadcast_to(qkv_b[rv][None, :], (P, HPC * D))
        ).astype(np.float32)
        woT = np.ascontiguousarray(out_w[:, hg * 512:(hg + 1) * 512].T).astype(bf)
        xT = np.ascontiguousarray(x[b].T).astype(bf)  # [C, T]
        in_maps.append({
            "xT": xT, "wT": wT, "bqk": bqk, "bv": bv,
            "embq": embq, "embk": embk, "woT": woT, "dmask": dmask,
        })
    return in_maps


_NC_CACHE = {}


def kernel(x, qkv_w, qkv_b, out_w, out_b):
    from concourse.bass_utils import run_bass_kernel_spmd

    x = np.asarray(x, dtype=np.float32)
    qkv_w = np.asarray(qkv_w, dtype=np.float32)
    qkv_b = np.asarray(qkv_b, dtype=np.float32)
    out_w = np.asarray(out_w, dtype=np.float32)
    out_b = np.asarray(out_b, dtype=np.float32)

    if "nc" not in _NC_CACHE:
        _NC_CACHE["nc"] = build_bass()
    nc = _NC_CACHE["nc"]

    in_maps = make_in_maps(x, qkv_w, qkv_b, out_w)
    res = run_bass_kernel_spmd(nc, in_maps, core_ids=list(range(8)))
    out = np.empty((B, T, C), dtype=np.float32)
    for b in range(B):
        acc = res.results[4 * b]["y"].astype(np.float32)
        for hg in range(1, 4):
            acc = acc + res.results[4 * b + hg]["y"].astype(np.float32)
        out[b] = acc + out_b[None, :]
    return out


# revision 41
# speedup vs baseline: 1.1574x; 1.1574x over previous
"""Multi-head causal attention (B=2, T=2048, C=2048, H=16, D=128) on 8 TRN2
NeuronCores.

Sharding: core = (batch, head_group): cores 0-3 -> batch 0, cores 4-7 ->
batch 1; head_group hg = core % 4 owns heads [4*hg, 4*hg+4).  Each core:
  qkv projection for its 4 heads (q/k transposed layout, v natural layout),
  multiplicative RoPE, causal attention with transposed scores, and the
  out-projection partial product over its 512 feature columns.
Host: per-batch sum of the 4 partial outputs + out bias (the "all-reduce").

The PE engine executes its queue in order, so idle gaps during the
exp-paced attention inner loop can only be filled at emission time.
Projection and out-projection work is therefore split into ~850ns units
(4 matmuls each, one PSUM bank, sequential k-accumulation) kept in a FIFO;
one unit (two for diagonal blocks, which also wait on the causal-mask
multiply) is emitted between each attention block's scores and AV matmuls,
hiding the exp latency.  The timing build software-pipelines the next
iteration's first projection chunk into the out-projection tail.  The
softmax denominator accumulates over four round-robin fp16 chains so each
DVE add depends only on the block four earlier, keeping it off the
per-block critical path.  Diagonal-strip matmuls/exps are narrowed to the
unmasked query columns.  Partial outputs are stored bf16.
"""

import math
from contextlib import ExitStack

import ml_dtypes
import numpy as np

import concourse.bass as bass
import concourse.tile as tile
from concourse import bacc, bass_isa, mybir

BF16 = mybir.dt.bfloat16
FP16 = mybir.dt.float16
FP32 = mybir.dt.float32
AF = mybir.ActivationFunctionType

B, T, C, H, D = 2, 2048, 2048, 16, 128
HPC = 4          # heads per core
P = 128
KO = C // P      # 16 k-tiles for the projection contraction
MCH = T // 512   # 4 m-chunks of 512 tokens
NJB = T // P     # 16 key blocks
NCI = T // 512   # 4 query chunks


def build_bass(iters: int = 1, timing_io: bool = False, no_pool: bool = False,
               only: str = "", force_steady: bool = False):
    del no_pool
    nc = bacc.Bacc("TRN2", target_bir_lowering=False, debug=False, num_devices=8)

    # timing_io: big tensors become Internal DRAM (contents garbage) so each
    # run ships only a tiny input/output over the axon tunnel
    def in_tensor(name, shape, dtype):
        if timing_io:
            return nc.dram_tensor(name, shape, dtype).ap()
        return nc.dram_tensor(name, shape, dtype, kind="ExternalInput").ap()

    xT_d = in_tensor("xT", [C, T], BF16)
    wT_d = in_tensor("wT", [C, 3 * HPC * D], BF16)
    bqk_d = nc.dram_tensor("bqk", [P, 2 * HPC], FP32, kind="ExternalInput").ap()
    bv_d = in_tensor("bv", [P, HPC * D], FP32)
    embq_d = in_tensor("embq", [P, T], BF16)
    embk_d = in_tensor("embk", [P, T], BF16)
    woT_d = in_tensor("woT", [HPC * D, C], BF16)
    dmask_d = in_tensor("dmask", [P, 4 * 512], FP16)
    if timing_io:
        y_d = nc.dram_tensor("y", [T, C], BF16).ap()
        yext_d = nc.dram_tensor("yext", [P, 512], BF16, kind="ExternalOutput").ap()
    else:
        y_d = nc.dram_tensor("y", [T, C], BF16, kind="ExternalOutput").ap()
        yext_d = None

    with tile.TileContext(nc) as tc, ExitStack() as ctx:
        cpool = ctx.enter_context(tc.tile_pool(name="consts", bufs=1))
        qk_pool = ctx.enter_context(tc.tile_pool(name="qk", bufs=1))
        v_pool = ctx.enter_context(tc.tile_pool(name="v", bufs=1))
        a_pool = ctx.enter_context(tc.tile_pool(name="aT", bufs=1))
        s_pool = ctx.enter_context(tc.tile_pool(name="es", bufs=8))
        r_pool = ctx.enter_context(tc.tile_pool(name="recip", bufs=8))
        rb_pool = ctx.enter_context(tc.tile_pool(name="recipb", bufs=3))
        y_pool = ctx.enter_context(tc.tile_pool(name="ys", bufs=6))
        # dedicated PSUM pools (8 banks total): filler (proj+outp) 2,
        # scores 2, AV accum 2, denominator 2
        psf_pool = ctx.enter_context(tc.tile_pool(name="psf", bufs=3, space="PSUM"))
        pss_pool = ctx.enter_context(tc.tile_pool(name="pss", bufs=3, space="PSUM"))
        pso_pool = ctx.enter_context(tc.tile_pool(name="pso", bufs=2, space="PSUM"))

        # const tiles (DMA emission deferred so xt chunk 0 leads the queue)
        w_sb = cpool.tile([P, KO, 3 * HPC * D], BF16)   # qkv weightsT
        woT_sb = cpool.tile([P, HPC, C], BF16)
        embq_sb = cpool.tile([P, T], BF16)
        embk_sb = cpool.tile([P, T], BF16)
        bqk_sb = cpool.tile([P, 2 * HPC], FP32)
        bv_sb = cpool.tile([P, HPC * D], FP32)
        # static double-buffered x tiles: chunk mc lives in xt_buf[mc % 2];
        # the DMA for chunk mc+2 is emitted at the start of attention chunk
        # mc, two chunks of lead so filler matmuls never wait on it
        xt_buf = [cpool.tile([P, KO, 512], BF16, name=f"xtbuf{i}")
                  for i in range(2)]
        dmask_sb = cpool.tile([P, 4 * 512], FP16)

        def load_consts():
            # smaller per-ko transfers so the first matmuls start early
            wTr = wT_d.rearrange("(ko p) n -> p ko n", p=P)
            for ko in range(KO):
                nc.sync.dma_start(w_sb[:, ko, :], wTr[:, ko, :])
            nc.sync.dma_start(bqk_sb[:], bqk_d)
            nc.sync.dma_start(bv_sb[:], bv_d)
            nc.sync.dma_start(embq_sb[:], embq_d)
            nc.sync.dma_start(embk_sb[:], embk_d)
            nc.sync.dma_start(woT_sb[:], woT_d.rearrange("(hh p) n -> p hh n", p=P))
            nc.sync.dma_start(dmask_sb[:], dmask_d)

        xTr = xT_d.rearrange("(ko p) m -> p ko m", p=P)

        KSUB = 4  # ko-tiles per filler unit (~850ns of PE work)

        def load_x(mc):
            nc.sync.dma_start(xt_buf[mc % 2][:], xTr[:, :, slice(mc * 512,
                                                                 (mc + 1) * 512)])

        def make_proj_units(mc, qkT, v_sb, it="", parts=(0, 1, 2)):
            """qkv projection for m-chunk mc as a list of small emit fns.
            parts selects passes: 0=q, 1=k, 2=v."""
            ms = slice(mc * 512, (mc + 1) * 512)
            xt = xt_buf[mc % 2]
            st = {}
            units = []

            for part in (p for p in (0, 1) if p in parts):
                for f in range(4):
                    fb = part * 4 + f
                    for k0 in range(0, KO, KSUB):
                        def u_qk(part=part, fb=fb, k0=k0):
                            if k0 == 0:
                                st["ps"] = psf_pool.tile(
                                    [P, 512], FP32, tag="mm",
                                    name=f"psqk{it}{mc}_{fb}")
                            ps = st["ps"]
                            for ko in range(k0, k0 + KSUB):
                                nc.tensor.matmul(
                                    ps[:],
                                    lhsT=w_sb[:, ko, fb * P:(fb + 1) * P],
                                    rhs=xt[:, ko, :],
                                    start=(ko == 0),
                                    stop=(ko == KO - 1),
                                )
                            if k0 + KSUB == KO:
                                # fused bias + multiplicative RoPE on DVE
                                emb = embq_sb if part == 0 else embk_sb
                                nc.vector.scalar_tensor_tensor(
                                    out=qkT[:, fb, ms], in0=ps[:],
                                    scalar=bqk_sb[:, fb:fb + 1],
                                    in1=emb[:, ms],
                                    op0=mybir.AluOpType.add,
                                    op1=mybir.AluOpType.mult,
                                )
                        units.append(u_qk)

            for s in range(4) if 2 in parts else ():
                for k0 in range(0, KO, KSUB):
                    def u_v(s=s, k0=k0):
                        if k0 == 0:
                            st["ps"] = psf_pool.tile(
                                [P, 512], FP32, tag="mm",
                                name=f"psv{it}{mc}_{s}")
                        ps = st["ps"]
                        for ko in range(k0, k0 + KSUB):
                            nc.tensor.matmul(
                                ps[:],
                                lhsT=xt[:, ko, s * P:(s + 1) * P],
                                rhs=w_sb[:, ko, 2 * HPC * D:3 * HPC * D],
                                start=(ko == 0),
                                stop=(ko == KO - 1),
                            )
                        if k0 + KSUB == KO:
                            nc.vector.tensor_add(
                                out=v_sb[:, mc * 4 + s, :], in0=ps[:],
                                in1=bv_sb[:],
                            )
                    units.append(u_v)
            return units

        def make_outp_units(mc, aT):
            """out-projection partial for token blocks of chunk mc."""
            units = []
            for mb in range(4 * mc, 4 * mc + 4):
                for cc in range(4):
                    def u(mb=mb, cc=cc):
                        ps = psf_pool.tile([P, 512], FP32, tag="mm",
                                           name=f"psy{mb}_{cc}")
                        for h in range(HPC):
                            nc.tensor.matmul(
                                ps[:],
                                lhsT=aT[:, h, mb * P:(mb + 1) * P],
                                rhs=woT_sb[:, h, cc * 512:(cc + 1) * 512],
                                start=(h == 0), stop=(h == HPC - 1),
                            )
                        ys = y_pool.tile([P, 512], BF16, tag="ys",
                                         name=f"ys{mb}_{cc}")
                        # alternate drain engine to split the load
                        if (mb + cc) % 2 == 0:
                            nc.vector.tensor_copy(out=ys[:], in_=ps[:])
                        else:
                            nc.scalar.copy(out=ys[:], in_=ps[:])
                        nc.sync.dma_start(
                            y_d[mb * P:(mb + 1) * P,
                                cc * 512:(cc + 1) * 512], ys[:]
                        )
                    units.append(u)
            return units

        def attn_chunk(h, ci, qkT, v_sb, aT, emit_filler):
            iss = slice(ci * 512, (ci + 1) * 512)
            njb = 4 * (ci + 1)
            ps_o = pso_pool.tile([P, 512], FP32, tag="mm", name=f"pso{h}_{ci}")
            # fp16 es + four interleaved denominator chains: block j's add
            # depends only on block j-4, so the DVE accumulation is off the
            # per-block critical path (and 16-bit ops get 2x DVE mode)
            nch = 4 if ci > 0 else 1
            acc = [r_pool.tile([P, 512], FP16, tag="acc",
                               name=f"acc{h}_{ci}_{c}") for c in range(nch)]
            es_t = [None] * njb

            def d0_of(jb):
                return max(jb * P - ci * 512, 0)

            def scores(jb):
                # scores + exp + causal mask for block jb; the matmul is
                # emitted one block ahead of the consuming AV so the exp
                # chain latency is hidden even when filler runs dry
                d0 = d0_of(jb)
                ps_s = pss_pool.tile([P, 512], FP32, tag="mm",
                                     name=f"pss{h}_{ci}_{jb}")
                nc.tensor.matmul(
                    ps_s[:, d0:],
                    lhsT=qkT[:, HPC + h, jb * P:(jb + 1) * P],
                    rhs=qkT[:, h, ci * 512 + d0:(ci + 1) * 512],
                    start=True, stop=True,
                )
                es = s_pool.tile([P, 512], FP16, tag="es", name=f"es{h}_{ci}_{jb}")
                es_t[jb] = es
                nc.scalar.activation(out=es[:, d0:], in_=ps_s[:, d0:], func=AF.Exp)
                if d0_of(jb + 1) > d0:   # diagonal block
                    dd = d0 // P
                    nc.vector.tensor_mul(
                        out=es[:, d0:], in0=es[:, d0:],
                        in1=dmask_sb[:, dd * 512 + d0:(dd + 1) * 512],
                    )

            scores(0)
            for jb in range(njb):
                d0 = d0_of(jb)
                diag = d0_of(jb + 1) > d0
                if jb + 1 < njb:
                    scores(jb + 1)
                emit_filler(2 if diag else 1, jb)
                es = es_t[jb]
                nc.tensor.matmul(
                    ps_o[:, d0:],
                    lhsT=v_sb[:, jb, h * D:(h + 1) * D],
                    rhs=es[:, d0:],
                    start=(jb == 0), stop=(jb == njb - 1),
                )
                # softmax denominator on DVE, round-robin across chains
                a = acc[jb % nch]
                if jb < nch:
                    nc.vector.tensor_copy(out=a[:], in_=es[:])
                else:
                    nc.vector.tensor_add(
                        out=a[:, d0:], in0=a[:, d0:], in1=es[:, d0:]
                    )
            if nch == 4:
                nc.vector.tensor_add(out=acc[0][:], in0=acc[0][:], in1=acc[1][:])
                nc.vector.tensor_add(out=acc[2][:], in0=acc[2][:], in1=acc[3][:])
                nc.vector.tensor_add(out=acc[0][:], in0=acc[0][:], in1=acc[2][:])
            rb = rb_pool.tile([P, 512], FP16, tag="rb", name=f"rb{h}_{ci}")
            rbr = rb_pool.tile([P, 512], FP32, tag="rbr", name=f"rbr{h}_{ci}")
            nc.gpsimd.partition_all_reduce(
                rb[:], acc[0][:], channels=P, reduce_op=bass_isa.ReduceOp.add
            )
            nc.vector.reciprocal(out=rbr[:], in_=rb[:])
            nc.vector.tensor_mul(out=aT[:, h, iss], in0=ps_o[:], in1=rbr[:])

        def body(emit_consts, steady):
            qkT = qk_pool.tile([P, 2 * HPC, T], BF16)    # q/k featT x tokens
            v_sb = v_pool.tile([P, NJB, HPC * D], BF16)  # tokens x v feats
            aT = a_pool.tile([P, HPC, T], BF16)          # attn outT per head

            filler = []   # list of (kind, fn); consumed selectively

            def add_filler(kind, units):
                filler.extend((kind, u) for u in units)

            def emit_filler(n, allowed=None):
                k = 0
                i = 0
                while k < n and i < len(filler):
                    kind, fn = filler[i]
                    if allowed is None or kind in allowed:
                        fn()
                        filler.pop(i)
                        k += 1
                    else:
                        i += 1
                return k

            def flush_filler(kinds=None):
                emit_filler(len(filler), kinds)

            if only in ("attn", "outp"):
                nc.vector.memset(qkT[:], 0.0)
                nc.vector.memset(v_sb[:], 0.0)
                if emit_consts:
                    load_consts()
            if only == "outp":
                nc.vector.memset(aT[:], 0.0)

            if only in ("", "proj") and not steady:
                load_x(0)
                load_x(1)
                if emit_consts:
                    load_consts()
                for u in make_proj_units(0, qkT, v_sb):
                    u()
            if only == "proj":
                for mc in range(1, MCH):
                    for u in make_proj_units(mc, qkT, v_sb):
                        u()
                # keep SBUF results live via a consumer
                nc.sync.dma_start(y_d[0:P, 0:512], qkT[:, 0, 0:512])
                nc.sync.dma_start(y_d[P:2 * P, 0:512], v_sb[:, 0, :])
                return
            if only == "outp":
                for mc in range(MCH):
                    for u in make_outp_units(mc, aT):
                        u()
                return

            # Filler budget: attn(mc) has 16*(mc+1) exp-latency slots.  Deferrable
            # work (out-projections, next iteration's q-projection) is saved
            # for the 64-slot attn(3); required projections (p{mc+1} before
            # attn(mc+1)) are flushed at their deadline.
            for mc in range(MCH):
                if only == "":
                    # x for chunk mc+2 (wrapping: next iteration's chunks in
                    # steady mode); its buffer's last reader finished in the
                    # previous attention chunk
                    if steady or mc + 2 < MCH:
                        load_x((mc + 2) % MCH)
                    if mc + 1 < MCH:
                        add_filler(f"p{mc + 1}",
                                   make_proj_units(mc + 1, qkT, v_sb))
                    if mc == 3 and steady:
                        # next iteration's q-projection: its only readers
                        # (attn(0) scores rhs) already ran this iteration
                        add_filler("p0q", make_proj_units(0, qkT, v_sb,
                                                          it="n", parts=(0,)))
                allowed = ({"p1"}, {"p2"}, {"p3", "o0"},
                           {"o0", "o1", "o2", "p0q"})[mc]
                for h in range(HPC):
                    attn_chunk(h, mc, qkT, v_sb, aT,
                               lambda n=1, jb=0, a=allowed: emit_filler(n, a))
                # next chunk's projection must finish before its attention
                if mc + 1 < MCH:
                    flush_filler({f"p{mc + 1}"})
                if only == "":
                    add_filler(f"o{mc}", make_outp_units(mc, aT))
            if steady:
                # k/v projections of the next iteration's chunk 0 must wait
                # for this iteration's last scores/AV reads, so they tail
                # (as filler inside attn(3) their DVE drains convoy behind
                # the mask/acc hops of the busiest head -- measured slower)
                add_filler("p0kv", make_proj_units(0, qkT, v_sb, it="n",
                                                   parts=(1, 2)))
            # projection tail first: dense and independent of the last head's
            # softmax, hiding the aT latency the out-projection tail waits on
            flush_filler({"p0q", "p0kv"})
            flush_filler()
            if only == "attn":
                nc.sync.dma_start(y_d[0:P, 0:512], aT[:, 0, 0:512])

        if iters == 1 and force_steady:
            load_consts()
            load_x(0)
            load_x(1)
            body(emit_consts=False, steady=True)
        elif iters == 1:
            body(emit_consts=True, steady=False)
        else:
            load_consts()
            load_x(0)
            load_x(1)
            with tc.For_i(0, iters, 1):
                body(emit_consts=False, steady=True)
        if yext_d is not None:
            nc.sync.dma_start(yext_d[:], y_d[0:P, 0:512])

    nc.compile()
    return nc


def _rope_emb():
    freqs = np.arange(0, D, 2, dtype=np.float32) / D
    angles = np.arange(T, dtype=np.float32)[:, None] * freqs[None, :]
    return np.concatenate([np.sin(angles), np.cos(angles)], axis=-1)  # [T, D]


def make_in_maps(x, qkv_w, qkv_b, out_w):
    bf = ml_dtypes.bfloat16
    emb = _rope_emb()
    embq = np.ascontiguousarray((emb / math.sqrt(D)).T).astype(bf)  # [D, T]
    embk = np.ascontiguousarray(emb.T).astype(bf)
    # causal masks for the 4 diagonal-block offsets:
    # m[jl, d*512+il] = 1 iff il >= jl + 128*d
    jl = np.arange(P)[:, None]
    il = np.arange(512)[None, :]
    dmask = np.concatenate(
        [(il >= jl + 128 * d) for d in range(4)], axis=1
    ).astype(np.float16)
    in_maps = []
    for core in range(8):
        b, hg = divmod(core, 4)
        rq = slice(hg * 512, (hg + 1) * 512)
        rk = slice(C + hg * 512, C + (hg + 1) * 512)
        rv = slice(2 * C + hg * 512, 2 * C + (hg + 1) * 512)
        w_slice = np.concatenate([qkv_w[rq], qkv_w[rk], qkv_w[rv]], axis=0)
        wT = np.ascontiguousarray(w_slice.T).astype(bf)  # [C, 1536]
        bq = qkv_b[rq].reshape(HPC, P).T  # [128, 4] per-block columns
        bk = qkv_b[rk].reshape(HPC, P).T
        bqk = np.ascontiguousarray(np.concatenate([bq, bk], axis=1)).astype(np.float32)
        bv = np.ascontiguousarray(
            np.broadcast_to(qkv_b[rv][None, :], (P, HPC * D))
        ).astype(np.float32)
        woT = np.ascontiguousarray(out_w[:, hg * 512:(hg + 1) * 512].T).astype(bf)
        xT = np.ascontiguousarray(x[b].T).astype(bf)  # [C, T]
        in_maps.append({
            "xT": xT, "wT": wT, "bqk": bqk, "bv": bv,
            "embq": embq, "embk": embk, "woT": woT, "dmask": dmask,
        })
    return in_maps


_NC_CACHE = {}


def kernel(x, qkv_w, qkv_b, out_w, out_b):
    from concourse.bass_utils import run_bass_kernel_spmd

    x = np.asarray(x, dtype=np.float32)
    qkv_w = np.asarray(qkv_w, dtype=np.float32)
    qkv_b = np.asarray(qkv_b, dtype=np.float32)
    out_w = np.asarray(out_w, dtype=np.float32)
    out_b = np.asarray(out_b, dtype=np.float32)

    if "nc" not in _NC_CACHE:
        _NC_CACHE["nc"] = build_bass()
    nc = _NC_CACHE["nc"]

    in_maps = make_in_maps(x, qkv_w, qkv_b, out_w)
    res = run_bass_kernel_spmd(nc, in_maps, core_ids=list(range(8)))
    out = np.empty((B, T, C), dtype=np.float32)
    for b in range(B):
        acc = res.results[4 * b]["y"].astype(np.float32)
        for hg in range(1, 4):
            acc = acc + res.results[4 * b + hg]["y"].astype(np.float32)
        out[b] = acc + out_b[None, :]
    return out
